# revision 10
# baseline (speedup 1.0000x reference)
"""CRF negative-log-likelihood loss kernel for Trainium2 (8 NeuronCores).

Strategy (v6: 165/91/91/165 split, order-pinned DVE pacing)
-----------------------------------------------------------
Data-parallel over the batch: 32 sequences -> 4 per core. The log-partition
chain Z = 1^T A_511 ... A_1 e_0 (A_t = diag(e_t) M, M = exp(T)) is split:

  - fwd vector chain  alpha_r = e_r * (M @ alpha_{r-1}),   steps [1, 165)
  - bwd vector chain  v_t = e_t * (M^T @ v_{t+1}),         steps [347, 512)
  - interior operators S1 [165,256) and S2 [256,347), computed as X = S^T
    via basis chains Y <- e_t * (M^T @ Y), Y_0 = diag(e_hi).

Both serial walls are latency-bound (~420ns/vec round from PE-SBUF +
DVE-PSUM access latencies, ~760ns/interior round), so V=165/C=91 balances
164 vec rounds against 91 interior rounds. Each super-round runs one
interior round (8 chains in 4 tiles: a [128,192] group scaled by one
broadcast-AP DVE tensor_tensor, a [128,64] tile scaled on Act) plus 1-2
vec rounds ([128,4] matmul + DVE TT). Order-only dependency edges pin the
DVE queue per super-round to [TT, scaleD, TT] so the scheduler cannot
de-pace the vec chain.

exp() prescaling (2^-7.5) is host-side; ln(Z) and the final subtract are
host-side. The combine uses matmul partition-offset placement (no
SBUF->SBUF partition-moving DMAs). Gold-path one-hot fp32 matmuls are
interleaved into the loop; inputs arrive as consolidated DMAs triggered
from parallel engine queues.
"""

import math

import numpy as np
from contextlib import ExitStack

B, T_LEN, L = 32, 512, 64
N_CORES = 8
BPC = B // N_CORES   # sequences per core
V = 165              # vec chain length: fwd [0,165), bwd [347,512)
C = 91               # interior chunk length: [165,256), [256,347)
PRE_BITS = 7.5
NVR = V - 1          # vec TT rounds (164)
NV = 4 * NVR         # ev cols
NSC = C - 1          # scaled interior rounds (90)
ND = 3 * NSC         # esd cols (DVE scales, seqs 0-2)
NA = NSC             # esa cols (Act scales, seq 3)
LN_OFF = float(T_LEN * PRE_BITS * math.log(2.0))

# consolidated bf16 input: wc | vs0 | wd | y0 (4 tiles) | ev
OFF_WC, OFF_VS0, OFF_WD, OFF_Y0 = 0, 128, 132, 260
OFF_EV = OFF_Y0 + 4 * L
NBF = OFF_EV + NV
# consolidated f32 input: esd | esa | mask
OFF_ESD, OFF_ESA, OFF_MASK = 0, ND, ND + NA
NF32 = OFF_MASK + L

_compiled = None

SKIP_LDW_WAIT_PASS = True


def _build_program():
    import concourse.bacc as bacc
    import concourse.tile as tile
    import concourse.mybir as mybir
    from concourse.alu_op_type import AluOpType
    from concourse.tile_rust import add_dep_helper

    f32 = mybir.dt.float32
    bf16 = mybir.dt.bfloat16
    Af = mybir.ActivationFunctionType

    nc = bacc.Bacc("TRN2", target_bir_lowering=False, debug=False,
                   num_devices=N_CORES)

    abf_d = nc.dram_tensor("abf", [128, NBF], bf16, kind="ExternalInput").ap()
    af32_d = nc.dram_tensor("af32", [128, NF32], f32,
                            kind="ExternalInput").ap()
    feats_d = nc.dram_tensor("feats", [BPC * T_LEN, L], f32,
                             kind="ExternalInput").ap()
    oh_d = nc.dram_tensor("oh", [BPC * (T_LEN + 1), L], f32,
                          kind="ExternalInput").ap()
    out_d = nc.dram_tensor("out", [1, 2 * BPC], f32, kind="ExternalOutput").ap()

    # vec rounds carried by super-round k (1..C): n_k in {1, 2};
    # PRE rounds run before the first super-round, during the DMA window
    PRE = 10
    NVS = NVR - PRE
    paces = [(NVS * k) // C - (NVS * (k - 1)) // C for k in range(1, C + 1)]
    assert sum(paces) == NVS

    with tile.TileContext(nc) as tc, ExitStack() as ctx:
        consts = ctx.enter_context(tc.tile_pool(name="consts", bufs=1))
        dpool = ctx.enter_context(tc.tile_pool(name="dstate", bufs=2))
        apool = ctx.enter_context(tc.tile_pool(name="astate", bufs=2))
        vpool = ctx.enter_context(tc.tile_pool(name="vstate", bufs=3))
        vtmp = ctx.enter_context(tc.tile_pool(name="vtmp", bufs=8))
        goldp = ctx.enter_context(tc.tile_pool(name="gold", bufs=6))
        pv = ctx.enter_context(tc.tile_pool(name="pv", bufs=2, space="PSUM"))
        pd = ctx.enter_context(tc.tile_pool(name="pd", bufs=2, space="PSUM"))
        pa = ctx.enter_context(tc.tile_pool(name="pa", bufs=2, space="PSUM"))
        pg = ctx.enter_context(tc.tile_pool(name="pg", bufs=1, space="PSUM"))
        pc = ctx.enter_context(tc.tile_pool(name="pc", bufs=1, space="PSUM"))

        # ---- consolidated operand tiles, DMA'd via parallel engine queues --
        abf = consts.tile([128, NBF], bf16)
        nc.sync.dma_start(out=abf[:, 0:OFF_Y0], in_=abf_d[:, 0:OFF_Y0])
        nc.gpsimd.dma_start(out=abf[:, OFF_Y0:OFF_EV],
                            in_=abf_d[:, OFF_Y0:OFF_EV])
        nc.sync.dma_start(out=abf[:, OFF_EV:NBF], in_=abf_d[:, OFF_EV:NBF])
        af32 = consts.tile([128, NF32], f32)
        nc.scalar.dma_start(out=af32[:], in_=af32_d)

        Wc = abf[:, OFF_WC:OFF_WC + 128]
        Wd = abf[:, OFF_WD:OFF_WD + 128]
        mask_t = af32[:, OFF_MASK:OFF_MASK + L]

        ones128 = consts.tile([128, 1], f32)
        nc.gpsimd.memset(ones128[:], 1.0)
        ones64 = consts.tile([L, 1], bf16)
        nc.gpsimd.memset(ones64[:], 1.0)
        Vt = consts.tile([128, BPC], f32)

        state = abf[:, OFF_VS0:OFF_VS0 + 4]
        Yd = abf[:, OFF_Y0:OFF_Y0 + 3 * L]
        Ya = abf[:, OFF_Y0 + 3 * L:OFF_Y0 + 4 * L]

        feats_bmaj = feats_d.rearrange("(b t) l -> b t l", b=BPC)

        # gold blocks interleaved on pace-1 super-rounds (slack there)
        slots = [k for k in range(3, C + 1) if paces[k - 1] == 1][:16]
        slots += [k for k in range(3, C + 1) if k not in slots][:16 - len(slots)]
        gold_at = {k: j for j, k in enumerate(sorted(slots[:16]))}
        gps = None

        r = 1
        for _p in range(PRE):
            q = pv.tile([128, 4], f32, tag="q")
            nc.tensor.matmul(q[:], lhsT=Wc, rhs=state, start=True, stop=True)
            ns = vpool.tile([128, 4], bf16, tag="vs")
            nc.vector.tensor_tensor(
                ns[:], q[:], abf[:, OFF_EV + 4 * (r - 1):OFF_EV + 4 * r],
                op=AluOpType.mult)
            state = ns[:]
            r += 1
        for k in range(1, C + 1):
            tta = None
            for j in range(paces[k - 1]):
                q = pv.tile([128, 4], f32, tag="q")
                mv = nc.tensor.matmul(q[:], lhsT=Wc, rhs=state,
                                      start=True, stop=True)
                ns = vpool.tile([128, 4], bf16, tag="vs")
                tt = nc.vector.tensor_tensor(
                    ns[:], q[:],
                    abf[:, OFF_EV + 4 * (r - 1):OFF_EV + 4 * r],
                    op=AluOpType.mult)
                state = ns[:]
                r += 1
                if j == 1:
                    # PE order: mmVb after mmD, mmA after mmVb
                    add_dep_helper(mv.ins, mdi.ins, sync=False,
                                   reason="PE pacing b")
                    add_dep_helper(mai.ins, mv.ins, sync=False,
                                   reason="PE pacing c")
                if j == 0:
                    tta = tt
                    pD = pd.tile([128, 3 * L], f32, tag="pd")
                    mdi = nc.tensor.matmul(pD[:], lhsT=Wd, rhs=Yd,
                                           start=True, stop=True)
                    add_dep_helper(mdi.ins, mv.ins, sync=False,
                                   reason="PE pacing a")
                    pA = pa.tile([128, L], f32, tag="pa")
                    mai = nc.tensor.matmul(pA[:], lhsT=Wd, rhs=Ya,
                                           start=True, stop=True)
                    if k <= C - 1:
                        c0 = OFF_ESD + 3 * (k - 1)
                        ynD = dpool.tile([128, 3 * L], bf16, tag="yd")
                        sdi = nc.vector.tensor_tensor(
                            ynD[:], pD[:],
                            af32[:, c0:c0 + 3].broadcast_to((128, 3, L)),
                            op=AluOpType.mult)
                        Yd = ynD[:]
                        ynA = apool.tile([128, L], bf16, tag="ya")
                        nc.scalar.activation(
                            ynA[:], pA[:], Af.Copy,
                            scale=af32[:, OFF_ESA + k - 1:OFF_ESA + k])
                        Ya = ynA[:]
                    else:
                        xd = vtmp.tile([128, 3 * L], bf16, tag="xd")
                        sdi = nc.vector.tensor_copy(xd[:], pD[:])
                        Yd = xd[:]
                        xa = vtmp.tile([128, L], bf16, tag="xa")
                        nc.scalar.activation(xa[:], pA[:], Af.Copy)
                        Ya = xa[:]
                    # DVE order: TTa before scaleD
                    add_dep_helper(sdi.ins, tta.ins, sync=False,
                                   reason="DVE pacing a")
                else:
                    # DVE order: TTb after scaleD
                    add_dep_helper(tt.ins, sdi.ins, sync=False,
                                   reason="DVE pacing b")
            # ---- interleaved gold block ----
            jb = gold_at.get(k)
            if jb is not None:
                s, c4 = divmod(jb, 4)
                o0 = s * (T_LEN + 1) + c4 * 128
                cat = goldp.tile([128, 128], f32, tag="cat")
                nc.sync.dma_start(out=cat[:, 0:L],
                                  in_=feats_bmaj[s, c4 * 128:(c4 + 1) * 128, :])
                nc.sync.dma_start(out=cat[:, L:128],
                                  in_=oh_d[o0 + 1:o0 + 129, :])
                ohp = goldp.tile([128, L], f32, tag="ohp")
                nc.sync.dma_start(out=ohp[:], in_=oh_d[o0:o0 + 128, :])
                if c4 == 0:
                    gps = pg.tile([128, L], f32, tag="tp")
                nc.tensor.matmul(gps[:], lhsT=cat[:], rhs=ohp[:],
                                 start=(c4 == 0), stop=(c4 == 3))
                if c4 == 3:
                    gsc = vtmp.tile([128, L], f32, tag="gsc")
                    nc.vector.tensor_mul(gsc[:], gps[:], mask_t)
                    nc.vector.tensor_reduce(Vt[:, s:s + 1], gsc[:],
                                            axis=mybir.AxisListType.X,
                                            op=AluOpType.add)

        # gold total: ready before the loop ends
        ores = vtmp.tile([1, 2 * BPC], f32, tag="ores")
        goldrow = pc.tile([1, BPC], f32, tag="c")
        nc.tensor.matmul(goldrow[:], lhsT=ones128[:, 0:1], rhs=Vt[:],
                         start=True, stop=True)
        nc.vector.tensor_copy(ores[:, BPC:2 * BPC], goldrow[:])

        # ---- combine: Z_s = u_s . (S2 S1 alpha)_s ----
        ups = pc.tile([L, 4], f32, tag="c")
        nc.tensor.matmul(ups[:], lhsT=Wd[:, L:128], rhs=state,
                         start=True, stop=True)
        usb = vtmp.tile([L, 4], bf16, tag="usb")
        nc.vector.tensor_copy(usb[:], ups[:])

        # z1_s = X1_s^T alpha_s -> partitions 64-127
        z1p = pc.tile([128, 4], f32, tag="c")
        for s in range(4):
            lhs = Yd[0:L, L * s:L * (s + 1)] if s < 3 else Ya[0:L, :]
            nc.tensor.matmul(z1p[L:128, s:s + 1], lhsT=lhs,
                             rhs=state[0:L, s:s + 1], start=True, stop=True)
        z1s = vtmp.tile([128, 4], bf16, tag="z1s")
        nc.vector.memset(z1s[0:L, :], 0.0)
        nc.vector.tensor_copy(z1s[L:128, :], z1p[L:128, :])

        # z2_s = X2_s^T z1_s via full-height lhsT (top half hits zeros)
        z2p = pc.tile([L, 4], f32, tag="c")
        for s in range(4):
            lhs2 = Yd[:, L * s:L * (s + 1)] if s < 3 else Ya[:, :]
            nc.tensor.matmul(z2p[:, s:s + 1], lhsT=lhs2,
                             rhs=z1s[:, s:s + 1], start=True, stop=True)
        g = vtmp.tile([L, 4], bf16, tag="g")
        nc.vector.tensor_tensor(g[:], z2p[:], usb[:], op=AluOpType.mult)
        zrow = pc.tile([1, 4], f32, tag="c")
        nc.tensor.matmul(zrow[:], lhsT=ones64[:, 0:1], rhs=g[:],
                         start=True, stop=True)
        nc.vector.tensor_copy(ores[:, 0:BPC], zrow[:])
        nc.sync.dma_start(out=out_d, in_=ores[:])

    import concourse.bacc as bacc2
    orig = bacc2.Bacc.move_matmul_waits_to_ldweights
    if SKIP_LDW_WAIT_PASS:
        bacc2.Bacc.move_matmul_waits_to_ldweights = lambda self: None
    try:
        nc.compile()
    finally:
        bacc2.Bacc.move_matmul_waits_to_ldweights = orig
    return nc


def _prep_in_maps(feats, tags, T):
    import ml_dtypes
    bf = ml_dtypes.bfloat16

    feats = np.ascontiguousarray(np.asarray(feats, dtype=np.float32))
    T_np = np.ascontiguousarray(np.asarray(T, dtype=np.float32))
    tags_np = np.asarray(tags).astype(np.int64)

    E = np.exp(feats - PRE_BITS * np.log(2.0)).astype(np.float32)
    M = np.exp(T_np)

    oh = np.zeros((B, T_LEN + 1, L), dtype=np.float32)
    oh[np.arange(B)[:, None], np.arange(T_LEN)[None, :], tags_np] = 1.0

    iL = np.arange(L)
    h1, h2 = V + C - 1, V + 2 * C - 1  # 255, 346
    in_maps = []
    for c in range(N_CORES):
        sl = slice(c * BPC, (c + 1) * BPC)
        Eb = E[sl]          # [4, 512, 64]
        fb = feats[sl]

        abf = np.zeros((128, NBF), dtype=np.float32)
        abf[0:L, OFF_WC:OFF_WC + L] = M.T
        abf[L:128, OFF_WC + L:OFF_WC + 128] = M
        abf[0:L, OFF_WD:OFF_WD + L] = M
        abf[L:128, OFF_WD + L:OFF_WD + 128] = M
        abf[0:L, OFF_VS0:OFF_VS0 + 4] = Eb[:, 0, :].T
        abf[L:128, OFF_VS0:OFF_VS0 + 4] = Eb[:, T_LEN - 1, :].T
        for s in range(4):
            abf[iL, OFF_Y0 + L * s + iL] = Eb[s, h1]
            abf[L + iL, OFF_Y0 + L * s + iL] = Eb[s, h2]
        abf[0:L, OFF_EV:] = Eb[:, 1:V, :].transpose(2, 1, 0).reshape(L, NV)
        abf[L:128, OFF_EV:] = Eb[:, T_LEN - 1:T_LEN - V:-1, :].transpose(
            2, 1, 0).reshape(L, NV)

        af32 = np.empty((128, NF32), dtype=np.float32)
        # esd col 3(kk-1)+s (s=0..2): [e_{h1-kk}(s); e_{h2-kk}(s)], kk=1..90
        af32[0:L, OFF_ESD:OFF_ESD + ND] = Eb[0:3, h1 - 1:h1 - C:-1, :].transpose(
            2, 1, 0).reshape(L, ND)
        af32[L:128, OFF_ESD:OFF_ESD + ND] = Eb[0:3, h2 - 1:h2 - C:-1, :].transpose(
            2, 1, 0).reshape(L, ND)
        af32[0:L, OFF_ESA:OFF_ESA + NA] = Eb[3, h1 - 1:h1 - C:-1, :].T
        af32[L:128, OFF_ESA:OFF_ESA + NA] = Eb[3, h2 - 1:h2 - C:-1, :].T
        af32[0:L, OFF_MASK:] = np.eye(L, dtype=np.float32)
        af32[L:128, OFF_MASK:] = T_np

        in_maps.append({
            "abf": abf.astype(bf),
            "af32": af32,
            "feats": np.ascontiguousarray(fb.reshape(BPC * T_LEN, L)),
            "oh": np.ascontiguousarray(oh[sl].reshape(BPC * (T_LEN + 1), L)),
        })
    return in_maps


def kernel(feats, tags, T):
    global _compiled
    from concourse.bass_utils import run_bass_kernel_spmd

    if _compiled is None:
        _compiled = _build_program()
    nc = _compiled

    in_maps = _prep_in_maps(feats, tags, T)
    res = run_bass_kernel_spmd(nc, in_maps, list(range(N_CORES)))
    outs = []
    for c in range(N_CORES):
        o = np.asarray(res.results[c]["out"], dtype=np.float64).reshape(2, BPC)
        outs.append(np.log(o[0]) + LN_OFF - o[1])
    return np.concatenate(outs).astype(np.float32)


# revision 11
# speedup vs baseline: 1.0232x; 1.0232x over previous
"""CRF negative-log-likelihood loss kernel for Trainium2 (8 NeuronCores).

Strategy (v6: 165/91/91/165 split, order-pinned DVE pacing)
-----------------------------------------------------------
Data-parallel over the batch: 32 sequences -> 4 per core. The log-partition
chain Z = 1^T A_511 ... A_1 e_0 (A_t = diag(e_t) M, M = exp(T)) is split:

  - fwd vector chain  alpha_r = e_r * (M @ alpha_{r-1}),   steps [1, 165)
  - bwd vector chain  v_t = e_t * (M^T @ v_{t+1}),         steps [347, 512)
  - interior operators S1 [165,256) and S2 [256,347), computed as X = S^T
    via basis chains Y <- e_t * (M^T @ Y), Y_0 = diag(e_hi).

Both serial walls are latency-bound (~420ns/vec round from PE-SBUF +
DVE-PSUM access latencies, ~760ns/interior round), so V=165/C=91 balances
164 vec rounds against 91 interior rounds. Each super-round runs one
interior round (8 chains in 4 tiles: a [128,192] group scaled by one
broadcast-AP DVE tensor_tensor, a [128,64] tile scaled on Act) plus 1-2
vec rounds ([128,4] matmul + DVE TT). Order-only dependency edges pin the
DVE queue per super-round to [TT, scaleD, TT] so the scheduler cannot
de-pace the vec chain.

exp() prescaling (2^-7.5) is host-side; ln(Z) and the final subtract are
host-side. The combine uses matmul partition-offset placement (no
SBUF->SBUF partition-moving DMAs). Gold-path one-hot fp32 matmuls are
interleaved into the loop; inputs arrive as consolidated DMAs triggered
from parallel engine queues.
"""

import math

import numpy as np
from contextlib import ExitStack

B, T_LEN, L = 32, 512, 64
N_CORES = 8
BPC = B // N_CORES   # sequences per core
V = 165              # vec chain length: fwd [0,165), bwd [347,512)
C = 91               # interior chunk length: [165,256), [256,347)
PRE_BITS = 7.5
NVR = V - 1          # vec TT rounds (164)
NV = 4 * NVR         # ev cols
NSC = C - 1          # scaled interior rounds (90)
ND = 3 * NSC         # esd cols (DVE scales, seqs 0-2)
NA = NSC             # esa cols (Act scales, seq 3)
LN_OFF = float(T_LEN * PRE_BITS * math.log(2.0))

# consolidated bf16 input: wc | vs0 | wd | y0 (4 tiles) | ev
OFF_WC, OFF_VS0, OFF_WD, OFF_Y0 = 0, 128, 132, 260
OFF_EV = OFF_Y0 + 4 * L
NBF = OFF_EV + NV
# consolidated f32 input: esd | esa | mask
OFF_ESD, OFF_ESA, OFF_MASK = 0, ND, ND + NA
NF32 = OFF_MASK + L

_compiled = None

SKIP_LDW_WAIT_PASS = True


def _build_program():
    import concourse.bacc as bacc
    import concourse.tile as tile
    import concourse.mybir as mybir
    from concourse.alu_op_type import AluOpType
    from concourse.tile_rust import add_dep_helper

    f32 = mybir.dt.float32
    bf16 = mybir.dt.bfloat16
    Af = mybir.ActivationFunctionType

    nc = bacc.Bacc("TRN2", target_bir_lowering=False, debug=False,
                   num_devices=N_CORES)

    abf_d = nc.dram_tensor("abf", [128, NBF], bf16, kind="ExternalInput").ap()
    af32_d = nc.dram_tensor("af32", [128, NF32], f32,
                            kind="ExternalInput").ap()
    feats_d = nc.dram_tensor("feats", [BPC * T_LEN, L], f32,
                             kind="ExternalInput").ap()
    oh_d = nc.dram_tensor("oh", [BPC * (T_LEN + 1), L], f32,
                          kind="ExternalInput").ap()
    out_d = nc.dram_tensor("out", [1, 2 * BPC], f32, kind="ExternalOutput").ap()

    # vec rounds carried by super-round k (1..C): n_k in {1, 2}
    paces = [(NVR * k) // C - (NVR * (k - 1)) // C for k in range(1, C + 1)]
    assert sum(paces) == NVR

    with tile.TileContext(nc) as tc, ExitStack() as ctx:
        consts = ctx.enter_context(tc.tile_pool(name="consts", bufs=1))
        dpool = ctx.enter_context(tc.tile_pool(name="dstate", bufs=2))
        apool = ctx.enter_context(tc.tile_pool(name="astate", bufs=2))
        vpool = ctx.enter_context(tc.tile_pool(name="vstate", bufs=3))
        vtmp = ctx.enter_context(tc.tile_pool(name="vtmp", bufs=8))
        goldp = ctx.enter_context(tc.tile_pool(name="gold", bufs=6))
        pv = ctx.enter_context(tc.tile_pool(name="pv", bufs=2, space="PSUM"))
        pd = ctx.enter_context(tc.tile_pool(name="pd", bufs=2, space="PSUM"))
        pa = ctx.enter_context(tc.tile_pool(name="pa", bufs=2, space="PSUM"))
        pg = ctx.enter_context(tc.tile_pool(name="pg", bufs=1, space="PSUM"))
        pc = ctx.enter_context(tc.tile_pool(name="pc", bufs=1, space="PSUM"))

        # ---- consolidated operand tiles, DMA'd via parallel engine queues --
        abf = consts.tile([128, NBF], bf16)
        nc.sync.dma_start(out=abf[:, 0:OFF_Y0], in_=abf_d[:, 0:OFF_Y0])
        nc.gpsimd.dma_start(out=abf[:, OFF_Y0:OFF_EV],
                            in_=abf_d[:, OFF_Y0:OFF_EV])
        nc.sync.dma_start(out=abf[:, OFF_EV:NBF], in_=abf_d[:, OFF_EV:NBF])
        af32 = consts.tile([128, NF32], f32)
        nc.scalar.dma_start(out=af32[:], in_=af32_d)

        Wc = abf[:, OFF_WC:OFF_WC + 128]
        Wd = abf[:, OFF_WD:OFF_WD + 128]
        mask_t = af32[:, OFF_MASK:OFF_MASK + L]

        ones128 = consts.tile([128, 1], f32)
        nc.gpsimd.memset(ones128[:], 1.0)
        ones64 = consts.tile([L, 1], bf16)
        nc.gpsimd.memset(ones64[:], 1.0)
        Vt = consts.tile([128, BPC], f32)

        state = abf[:, OFF_VS0:OFF_VS0 + 4]
        Yd = abf[:, OFF_Y0:OFF_Y0 + 3 * L]
        Ya = abf[:, OFF_Y0 + 3 * L:OFF_Y0 + 4 * L]

        feats_bmaj = feats_d.rearrange("(b t) l -> b t l", b=BPC)

        # gold blocks interleaved at super-rounds 5,8,...,50
        gold_at = {5 + 3 * j: j for j in range(16)}
        gps = None

        r = 1
        for k in range(1, C + 1):
            tta = None
            for j in range(paces[k - 1]):
                q = pv.tile([128, 4], f32, tag="q")
                mv = nc.tensor.matmul(q[:], lhsT=Wc, rhs=state,
                                      start=True, stop=True)
                ns = vpool.tile([128, 4], bf16, tag="vs")
                tt = nc.vector.tensor_tensor(
                    ns[:], q[:],
                    abf[:, OFF_EV + 4 * (r - 1):OFF_EV + 4 * r],
                    op=AluOpType.mult)
                state = ns[:]
                r += 1
                if j == 1:
                    # PE order: mmVb after mmD, mmA after mmVb
                    add_dep_helper(mv.ins, mdi.ins, sync=False,
                                   reason="PE pacing b")
                    add_dep_helper(mai.ins, mv.ins, sync=False,
                                   reason="PE pacing c")
                if j == 0:
                    tta = tt
                    pD = pd.tile([128, 3 * L], f32, tag="pd")
                    mdi = nc.tensor.matmul(pD[:], lhsT=Wd, rhs=Yd,
                                           start=True, stop=True)
                    add_dep_helper(mdi.ins, mv.ins, sync=False,
                                   reason="PE pacing a")
                    pA = pa.tile([128, L], f32, tag="pa")
                    mai = nc.tensor.matmul(pA[:], lhsT=Wd, rhs=Ya,
                                           start=True, stop=True)
                    if k <= C - 1:
                        c0 = OFF_ESD + 3 * (k - 1)
                        ynD = dpool.tile([128, 3 * L], bf16, tag="yd")
                        sdi = nc.vector.tensor_tensor(
                            ynD[:], pD[:],
                            af32[:, c0:c0 + 3].broadcast_to((128, 3, L)),
                            op=AluOpType.mult)
                        Yd = ynD[:]
                        ynA = apool.tile([128, L], bf16, tag="ya")
                        nc.scalar.activation(
                            ynA[:], pA[:], Af.Copy,
                            scale=af32[:, OFF_ESA + k - 1:OFF_ESA + k])
                        Ya = ynA[:]
                    else:
                        xd = vtmp.tile([128, 3 * L], bf16, tag="xd")
                        sdi = nc.vector.tensor_copy(xd[:], pD[:])
                        Yd = xd[:]
                        xa = vtmp.tile([128, L], bf16, tag="xa")
                        nc.scalar.activation(xa[:], pA[:], Af.Copy)
                        Ya = xa[:]
                    # DVE order: TTa before scaleD
                    add_dep_helper(sdi.ins, tta.ins, sync=False,
                                   reason="DVE pacing a")
                else:
                    # DVE order: TTb after scaleD
                    add_dep_helper(tt.ins, sdi.ins, sync=False,
                                   reason="DVE pacing b")
            # ---- interleaved gold block ----
            jb = gold_at.get(k)
            if jb is not None:
                s, c4 = divmod(jb, 4)
                o0 = s * (T_LEN + 1) + c4 * 128
                cat = goldp.tile([128, 128], f32, tag="cat")
                nc.sync.dma_start(out=cat[:, 0:L],
                                  in_=feats_bmaj[s, c4 * 128:(c4 + 1) * 128, :])
                nc.sync.dma_start(out=cat[:, L:128],
                                  in_=oh_d[o0 + 1:o0 + 129, :])
                ohp = goldp.tile([128, L], f32, tag="ohp")
                nc.sync.dma_start(out=ohp[:], in_=oh_d[o0:o0 + 128, :])
                if c4 == 0:
                    gps = pg.tile([128, L], f32, tag="tp")
                nc.tensor.matmul(gps[:], lhsT=cat[:], rhs=ohp[:],
                                 start=(c4 == 0), stop=(c4 == 3))
                if c4 == 3:
                    gsc = vtmp.tile([128, L], f32, tag="gsc")
                    nc.vector.tensor_mul(gsc[:], gps[:], mask_t)
                    nc.vector.tensor_reduce(Vt[:, s:s + 1], gsc[:],
                                            axis=mybir.AxisListType.X,
                                            op=AluOpType.add)

        # gold total: ready before the loop ends
        ores = vtmp.tile([1, 2 * BPC], f32, tag="ores")
        goldrow = pc.tile([1, BPC], f32, tag="c")
        nc.tensor.matmul(goldrow[:], lhsT=ones128[:, 0:1], rhs=Vt[:],
                         start=True, stop=True)
        nc.vector.tensor_copy(ores[:, BPC:2 * BPC], goldrow[:])

        # ---- combine: Z_s = u_s . (S2 S1 alpha)_s ----
        ups = pc.tile([L, 4], f32, tag="c")
        nc.tensor.matmul(ups[:], lhsT=Wd[:, L:128], rhs=state,
                         start=True, stop=True)
        usb = vtmp.tile([L, 4], bf16, tag="usb")
        nc.vector.tensor_copy(usb[:], ups[:])

        # z1_s = X1_s^T alpha_s -> partitions 64-127
        z1p = pc.tile([128, 4], f32, tag="c")
        for s in range(4):
            lhs = Yd[0:L, L * s:L * (s + 1)] if s < 3 else Ya[0:L, :]
            nc.tensor.matmul(z1p[L:128, s:s + 1], lhsT=lhs,
                             rhs=state[0:L, s:s + 1], start=True, stop=True)
        z1s = vtmp.tile([128, 4], bf16, tag="z1s")
        nc.vector.memset(z1s[0:L, :], 0.0)
        nc.vector.tensor_copy(z1s[L:128, :], z1p[L:128, :])

        # z2_s = X2_s^T z1_s via full-height lhsT (top half hits zeros)
        z2p = pc.tile([L, 4], f32, tag="c")
        for s in range(4):
            lhs2 = Yd[:, L * s:L * (s + 1)] if s < 3 else Ya[:, :]
            nc.tensor.matmul(z2p[:, s:s + 1], lhsT=lhs2,
                             rhs=z1s[:, s:s + 1], start=True, stop=True)
        g = vtmp.tile([L, 4], bf16, tag="g")
        nc.vector.tensor_tensor(g[:], z2p[:], usb[:], op=AluOpType.mult)
        zrow = pc.tile([1, 4], f32, tag="c")
        nc.tensor.matmul(zrow[:], lhsT=ones64[:, 0:1], rhs=g[:],
                         start=True, stop=True)
        nc.vector.tensor_copy(ores[:, 0:BPC], zrow[:])
        nc.sync.dma_start(out=out_d, in_=ores[:])

    import concourse.bacc as bacc2
    orig = bacc2.Bacc.move_matmul_waits_to_ldweights
    if SKIP_LDW_WAIT_PASS:
        bacc2.Bacc.move_matmul_waits_to_ldweights = lambda self: None
    try:
        nc.compile()
    finally:
        bacc2.Bacc.move_matmul_waits_to_ldweights = orig
    return nc


def _prep_in_maps(feats, tags, T):
    import ml_dtypes
    bf = ml_dtypes.bfloat16

    feats = np.ascontiguousarray(np.asarray(feats, dtype=np.float32))
    T_np = np.ascontiguousarray(np.asarray(T, dtype=np.float32))
    tags_np = np.asarray(tags).astype(np.int64)

    E = np.exp(feats - PRE_BITS * np.log(2.0)).astype(np.float32)
    M = np.exp(T_np)

    oh = np.zeros((B, T_LEN + 1, L), dtype=np.float32)
    oh[np.arange(B)[:, None], np.arange(T_LEN)[None, :], tags_np] = 1.0

    iL = np.arange(L)
    h1, h2 = V + C - 1, V + 2 * C - 1  # 255, 346
    in_maps = []
    for c in range(N_CORES):
        sl = slice(c * BPC, (c + 1) * BPC)
        Eb = E[sl]          # [4, 512, 64]
        fb = feats[sl]

        abf = np.zeros((128, NBF), dtype=np.float32)
        abf[0:L, OFF_WC:OFF_WC + L] = M.T
        abf[L:128, OFF_WC + L:OFF_WC + 128] = M
        abf[0:L, OFF_WD:OFF_WD + L] = M
        abf[L:128, OFF_WD + L:OFF_WD + 128] = M
        abf[0:L, OFF_VS0:OFF_VS0 + 4] = Eb[:, 0, :].T
        abf[L:128, OFF_VS0:OFF_VS0 + 4] = Eb[:, T_LEN - 1, :].T
        for s in range(4):
            abf[iL, OFF_Y0 + L * s + iL] = Eb[s, h1]
            abf[L + iL, OFF_Y0 + L * s + iL] = Eb[s, h2]
        abf[0:L, OFF_EV:] = Eb[:, 1:V, :].transpose(2, 1, 0).reshape(L, NV)
        abf[L:128, OFF_EV:] = Eb[:, T_LEN - 1:T_LEN - V:-1, :].transpose(
            2, 1, 0).reshape(L, NV)

        af32 = np.empty((128, NF32), dtype=np.float32)
        # esd col 3(kk-1)+s (s=0..2): [e_{h1-kk}(s); e_{h2-kk}(s)], kk=1..90
        af32[0:L, OFF_ESD:OFF_ESD + ND] = Eb[0:3, h1 - 1:h1 - C:-1, :].transpose(
            2, 1, 0).reshape(L, ND)
        af32[L:128, OFF_ESD:OFF_ESD + ND] = Eb[0:3, h2 - 1:h2 - C:-1, :].transpose(
            2, 1, 0).reshape(L, ND)
        af32[0:L, OFF_ESA:OFF_ESA + NA] = Eb[3, h1 - 1:h1 - C:-1, :].T
        af32[L:128, OFF_ESA:OFF_ESA + NA] = Eb[3, h2 - 1:h2 - C:-1, :].T
        af32[0:L, OFF_MASK:] = np.eye(L, dtype=np.float32)
        af32[L:128, OFF_MASK:] = T_np

        in_maps.append({
            "abf": abf.astype(bf),
            "af32": af32,
            "feats": np.ascontiguousarray(fb.reshape(BPC * T_LEN, L)),
            "oh": np.ascontiguousarray(oh[sl].reshape(BPC * (T_LEN + 1), L)),
        })
    return in_maps


def kernel(feats, tags, T):
    global _compiled
    from concourse.bass_utils import run_bass_kernel_spmd

    if _compiled is None:
        _compiled = _build_program()
    nc = _compiled

    in_maps = _prep_in_maps(feats, tags, T)
    res = run_bass_kernel_spmd(nc, in_maps, list(range(N_CORES)))
    outs = []
    for c in range(N_CORES):
        o = np.asarray(res.results[c]["out"], dtype=np.float64).reshape(2, BPC)
        outs.append(np.log(o[0]) + LN_OFF - o[1])
    return np.concatenate(outs).astype(np.float32)


# revision 12
# speedup vs baseline: 1.0621x; 1.0380x over previous
"""CRF negative-log-likelihood loss kernel for Trainium2 (8 NeuronCores).

Strategy (v6: 165/91/91/165 split, order-pinned DVE pacing)
-----------------------------------------------------------
Data-parallel over the batch: 32 sequences -> 4 per core. The log-partition
chain Z = 1^T A_511 ... A_1 e_0 (A_t = diag(e_t) M, M = exp(T)) is split:

  - fwd vector chain  alpha_r = e_r * (M @ alpha_{r-1}),   steps [1, 165)
  - bwd vector chain  v_t = e_t * (M^T @ v_{t+1}),         steps [347, 512)
  - interior operators S1 [165,256) and S2 [256,347), computed as X = S^T
    via basis chains Y <- e_t * (M^T @ Y), Y_0 = diag(e_hi).

Both serial walls are latency-bound (~420ns/vec round from PE-SBUF +
DVE-PSUM access latencies, ~760ns/interior round), so V=165/C=91 balances
164 vec rounds against 91 interior rounds. Each super-round runs one
interior round (8 chains in 4 tiles: a [128,192] group scaled by one
broadcast-AP DVE tensor_tensor, a [128,64] tile scaled on Act) plus 1-2
vec rounds ([128,4] matmul + DVE TT). Order-only dependency edges pin the
DVE queue per super-round to [TT, scaleD, TT] so the scheduler cannot
de-pace the vec chain.

exp() prescaling (2^-7.5) is host-side; ln(Z) and the final subtract are
host-side. The combine uses matmul partition-offset placement (no
SBUF->SBUF partition-moving DMAs). Gold-path one-hot fp32 matmuls are
interleaved into the loop; inputs arrive as consolidated DMAs triggered
from parallel engine queues.
"""

import math

import numpy as np
from contextlib import ExitStack

B, T_LEN, L = 32, 512, 64
N_CORES = 8
BPC = B // N_CORES   # sequences per core
V = 171              # vec chain length: fwd [0,171), bwd [341,512)
C = 85               # interior chunk length: [171,256), [256,341)
PRE_BITS = 7.5
NVR = V - 1          # vec TT rounds (164)
NV = 4 * NVR         # ev cols
NSC = C - 1          # scaled interior rounds (90)
ND = 3 * NSC         # esd cols (DVE scales, seqs 0-2)
NA = NSC             # esa cols (Act scales, seq 3)
LN_OFF = float(T_LEN * PRE_BITS * math.log(2.0))

# consolidated bf16 input: wc | vs0 | wd | y0 (4 tiles) | ev
OFF_WC, OFF_VS0, OFF_WD, OFF_Y0 = 0, 128, 132, 260
OFF_EV = OFF_Y0 + 4 * L
NBF = OFF_EV + NV
# consolidated f32 input: esd | esa | mask
OFF_ESD, OFF_ESA, OFF_MASK = 0, ND, ND + NA
NF32 = OFF_MASK + L

_compiled = None

SKIP_LDW_WAIT_PASS = True


def _build_program():
    import concourse.bacc as bacc
    import concourse.tile as tile
    import concourse.mybir as mybir
    from concourse.alu_op_type import AluOpType
    from concourse.tile_rust import add_dep_helper

    f32 = mybir.dt.float32
    bf16 = mybir.dt.bfloat16
    Af = mybir.ActivationFunctionType

    nc = bacc.Bacc("TRN2", target_bir_lowering=False, debug=False,
                   num_devices=N_CORES)

    abf_d = nc.dram_tensor("abf", [128, NBF], bf16, kind="ExternalInput").ap()
    af32_d = nc.dram_tensor("af32", [128, NF32], f32,
                            kind="ExternalInput").ap()
    feats_d = nc.dram_tensor("feats", [BPC * T_LEN, L], f32,
                             kind="ExternalInput").ap()
    oh_d = nc.dram_tensor("oh", [BPC * (T_LEN + 1), L], f32,
                          kind="ExternalInput").ap()
    out_d = nc.dram_tensor("out", [1, 2 * BPC], f32, kind="ExternalOutput").ap()

    # vec rounds carried by super-round k (1..C): n_k in {1, 2}
    paces = [(NVR * k) // C - (NVR * (k - 1)) // C for k in range(1, C + 1)]
    assert sum(paces) == NVR

    with tile.TileContext(nc) as tc, ExitStack() as ctx:
        consts = ctx.enter_context(tc.tile_pool(name="consts", bufs=1))
        dpool = ctx.enter_context(tc.tile_pool(name="dstate", bufs=2))
        apool = ctx.enter_context(tc.tile_pool(name="astate", bufs=2))
        vpool = ctx.enter_context(tc.tile_pool(name="vstate", bufs=3))
        vtmp = ctx.enter_context(tc.tile_pool(name="vtmp", bufs=8))
        goldp = ctx.enter_context(tc.tile_pool(name="gold", bufs=6))
        pv = ctx.enter_context(tc.tile_pool(name="pv", bufs=2, space="PSUM"))
        pd = ctx.enter_context(tc.tile_pool(name="pd", bufs=2, space="PSUM"))
        pa = ctx.enter_context(tc.tile_pool(name="pa", bufs=2, space="PSUM"))
        pg = ctx.enter_context(tc.tile_pool(name="pg", bufs=1, space="PSUM"))
        pc = ctx.enter_context(tc.tile_pool(name="pc", bufs=1, space="PSUM"))

        # ---- consolidated operand tiles, DMA'd via parallel engine queues --
        abf = consts.tile([128, NBF], bf16)
        nc.sync.dma_start(out=abf[:, 0:OFF_Y0], in_=abf_d[:, 0:OFF_Y0])
        nc.gpsimd.dma_start(out=abf[:, OFF_Y0:OFF_EV],
                            in_=abf_d[:, OFF_Y0:OFF_EV])
        nc.sync.dma_start(out=abf[:, OFF_EV:NBF], in_=abf_d[:, OFF_EV:NBF])
        af32 = consts.tile([128, NF32], f32)
        nc.scalar.dma_start(out=af32[:], in_=af32_d)

        Wc = abf[:, OFF_WC:OFF_WC + 128]
        Wd = abf[:, OFF_WD:OFF_WD + 128]
        mask_t = af32[:, OFF_MASK:OFF_MASK + L]

        ones128 = consts.tile([128, 1], f32)
        nc.gpsimd.memset(ones128[:], 1.0)
        ones64 = consts.tile([L, 1], bf16)
        nc.gpsimd.memset(ones64[:], 1.0)
        Vt = consts.tile([128, BPC], f32)

        state = abf[:, OFF_VS0:OFF_VS0 + 4]
        Yd = abf[:, OFF_Y0:OFF_Y0 + 3 * L]
        Ya = abf[:, OFF_Y0 + 3 * L:OFF_Y0 + 4 * L]

        feats_bmaj = feats_d.rearrange("(b t) l -> b t l", b=BPC)

        # gold blocks interleaved at super-rounds 5,8,...,50
        gold_at = {5 + 3 * j: j for j in range(16)}
        gps = None

        r = 1
        for k in range(1, C + 1):
            tta = None
            for j in range(paces[k - 1]):
                q = pv.tile([128, 4], f32, tag="q")
                mv = nc.tensor.matmul(q[:], lhsT=Wc, rhs=state,
                                      start=True, stop=True)
                ns = vpool.tile([128, 4], bf16, tag="vs")
                tt = nc.vector.tensor_tensor(
                    ns[:], q[:],
                    abf[:, OFF_EV + 4 * (r - 1):OFF_EV + 4 * r],
                    op=AluOpType.mult)
                state = ns[:]
                r += 1
                if j == 1:
                    # PE order: mmVb after mmD, mmA after mmVb
                    add_dep_helper(mv.ins, mdi.ins, sync=False,
                                   reason="PE pacing b")
                    add_dep_helper(mai.ins, mv.ins, sync=False,
                                   reason="PE pacing c")
                if j == 0:
                    tta = tt
                    pD = pd.tile([128, 3 * L], f32, tag="pd")
                    mdi = nc.tensor.matmul(pD[:], lhsT=Wd, rhs=Yd,
                                           start=True, stop=True)
                    add_dep_helper(mdi.ins, mv.ins, sync=False,
                                   reason="PE pacing a")
                    pA = pa.tile([128, L], f32, tag="pa")
                    mai = nc.tensor.matmul(pA[:], lhsT=Wd, rhs=Ya,
                                           start=True, stop=True)
                    if k <= C - 1:
                        c0 = OFF_ESD + 3 * (k - 1)
                        ynD = dpool.tile([128, 3 * L], bf16, tag="yd")
                        sdi = nc.vector.tensor_tensor(
                            ynD[:], pD[:],
                            af32[:, c0:c0 + 3].broadcast_to((128, 3, L)),
                            op=AluOpType.mult)
                        Yd = ynD[:]
                        ynA = apool.tile([128, L], bf16, tag="ya")
                        nc.scalar.activation(
                            ynA[:], pA[:], Af.Copy,
                            scale=af32[:, OFF_ESA + k - 1:OFF_ESA + k])
                        Ya = ynA[:]
                    else:
                        xd = vtmp.tile([128, 3 * L], bf16, tag="xd")
                        sdi = nc.vector.tensor_copy(xd[:], pD[:])
                        Yd = xd[:]
                        xa = vtmp.tile([128, L], bf16, tag="xa")
                        nc.scalar.activation(xa[:], pA[:], Af.Copy)
                        Ya = xa[:]
                    # DVE order: TTa before scaleD
                    add_dep_helper(sdi.ins, tta.ins, sync=False,
                                   reason="DVE pacing a")
                else:
                    # DVE order: TTb after scaleD
                    add_dep_helper(tt.ins, sdi.ins, sync=False,
                                   reason="DVE pacing b")
            # ---- interleaved gold block ----
            jb = gold_at.get(k)
            if jb is not None:
                s, c4 = divmod(jb, 4)
                o0 = s * (T_LEN + 1) + c4 * 128
                cat = goldp.tile([128, 128], f32, tag="cat")
                nc.sync.dma_start(out=cat[:, 0:L],
                                  in_=feats_bmaj[s, c4 * 128:(c4 + 1) * 128, :])
                nc.sync.dma_start(out=cat[:, L:128],
                                  in_=oh_d[o0 + 1:o0 + 129, :])
                ohp = goldp.tile([128, L], f32, tag="ohp")
                nc.sync.dma_start(out=ohp[:], in_=oh_d[o0:o0 + 128, :])
                if c4 == 0:
                    gps = pg.tile([128, L], f32, tag="tp")
                nc.tensor.matmul(gps[:], lhsT=cat[:], rhs=ohp[:],
                                 start=(c4 == 0), stop=(c4 == 3))
                if c4 == 3:
                    gsc = vtmp.tile([128, L], f32, tag="gsc")
                    nc.vector.tensor_mul(gsc[:], gps[:], mask_t)
                    nc.vector.tensor_reduce(Vt[:, s:s + 1], gsc[:],
                                            axis=mybir.AxisListType.X,
                                            op=AluOpType.add)

        # gold total: ready before the loop ends
        ores = vtmp.tile([1, 2 * BPC], f32, tag="ores")
        goldrow = pc.tile([1, BPC], f32, tag="c")
        nc.tensor.matmul(goldrow[:], lhsT=ones128[:, 0:1], rhs=Vt[:],
                         start=True, stop=True)
        nc.vector.tensor_copy(ores[:, BPC:2 * BPC], goldrow[:])

        # ---- combine: Z_s = u_s . (S2 S1 alpha)_s ----
        ups = pc.tile([L, 4], f32, tag="c")
        nc.tensor.matmul(ups[:], lhsT=Wd[:, L:128], rhs=state,
                         start=True, stop=True)
        usb = vtmp.tile([L, 4], bf16, tag="usb")
        nc.vector.tensor_copy(usb[:], ups[:])

        # z1_s = X1_s^T alpha_s -> partitions 64-127
        z1p = pc.tile([128, 4], f32, tag="c")
        for s in range(4):
            lhs = Yd[0:L, L * s:L * (s + 1)] if s < 3 else Ya[0:L, :]
            nc.tensor.matmul(z1p[L:128, s:s + 1], lhsT=lhs,
                             rhs=state[0:L, s:s + 1], start=True, stop=True)
        z1s = vtmp.tile([128, 4], bf16, tag="z1s")
        nc.vector.memset(z1s[0:L, :], 0.0)
        nc.vector.tensor_copy(z1s[L:128, :], z1p[L:128, :])

        # z2_s = X2_s^T z1_s via full-height lhsT (top half hits zeros)
        z2p = pc.tile([L, 4], f32, tag="c")
        for s in range(4):
            lhs2 = Yd[:, L * s:L * (s + 1)] if s < 3 else Ya[:, :]
            nc.tensor.matmul(z2p[:, s:s + 1], lhsT=lhs2,
                             rhs=z1s[:, s:s + 1], start=True, stop=True)
        g = vtmp.tile([L, 4], bf16, tag="g")
        nc.vector.tensor_tensor(g[:], z2p[:], usb[:], op=AluOpType.mult)
        zrow = pc.tile([1, 4], f32, tag="c")
        nc.tensor.matmul(zrow[:], lhsT=ones64[:, 0:1], rhs=g[:],
                         start=True, stop=True)
        nc.vector.tensor_copy(ores[:, 0:BPC], zrow[:])
        nc.sync.dma_start(out=out_d, in_=ores[:])

    import concourse.bacc as bacc2
    orig = bacc2.Bacc.move_matmul_waits_to_ldweights
    if SKIP_LDW_WAIT_PASS:
        bacc2.Bacc.move_matmul_waits_to_ldweights = lambda self: None
    try:
        nc.compile()
    finally:
        bacc2.Bacc.move_matmul_waits_to_ldweights = orig
    return nc


def _prep_in_maps(feats, tags, T):
    import ml_dtypes
    bf = ml_dtypes.bfloat16

    feats = np.ascontiguousarray(np.asarray(feats, dtype=np.float32))
    T_np = np.ascontiguousarray(np.asarray(T, dtype=np.float32))
    tags_np = np.asarray(tags).astype(np.int64)

    E = np.exp(feats - PRE_BITS * np.log(2.0)).astype(np.float32)
    M = np.exp(T_np)

    oh = np.zeros((B, T_LEN + 1, L), dtype=np.float32)
    oh[np.arange(B)[:, None], np.arange(T_LEN)[None, :], tags_np] = 1.0

    iL = np.arange(L)
    h1, h2 = V + C - 1, V + 2 * C - 1  # 255, 346
    in_maps = []
    for c in range(N_CORES):
        sl = slice(c * BPC, (c + 1) * BPC)
        Eb = E[sl]          # [4, 512, 64]
        fb = feats[sl]

        abf = np.zeros((128, NBF), dtype=np.float32)
        abf[0:L, OFF_WC:OFF_WC + L] = M.T
        abf[L:128, OFF_WC + L:OFF_WC + 128] = M
        abf[0:L, OFF_WD:OFF_WD + L] = M
        abf[L:128, OFF_WD + L:OFF_WD + 128] = M
        abf[0:L, OFF_VS0:OFF_VS0 + 4] = Eb[:, 0, :].T
        abf[L:128, OFF_VS0:OFF_VS0 + 4] = Eb[:, T_LEN - 1, :].T
        for s in range(4):
            abf[iL, OFF_Y0 + L * s + iL] = Eb[s, h1]
            abf[L + iL, OFF_Y0 + L * s + iL] = Eb[s, h2]
        abf[0:L, OFF_EV:] = Eb[:, 1:V, :].transpose(2, 1, 0).reshape(L, NV)
        abf[L:128, OFF_EV:] = Eb[:, T_LEN - 1:T_LEN - V:-1, :].transpose(
            2, 1, 0).reshape(L, NV)

        af32 = np.empty((128, NF32), dtype=np.float32)
        # esd col 3(kk-1)+s (s=0..2): [e_{h1-kk}(s); e_{h2-kk}(s)], kk=1..90
        af32[0:L, OFF_ESD:OFF_ESD + ND] = Eb[0:3, h1 - 1:h1 - C:-1, :].transpose(
            2, 1, 0).reshape(L, ND)
        af32[L:128, OFF_ESD:OFF_ESD + ND] = Eb[0:3, h2 - 1:h2 - C:-1, :].transpose(
            2, 1, 0).reshape(L, ND)
        af32[0:L, OFF_ESA:OFF_ESA + NA] = Eb[3, h1 - 1:h1 - C:-1, :].T
        af32[L:128, OFF_ESA:OFF_ESA + NA] = Eb[3, h2 - 1:h2 - C:-1, :].T
        af32[0:L, OFF_MASK:] = np.eye(L, dtype=np.float32)
        af32[L:128, OFF_MASK:] = T_np

        in_maps.append({
            "abf": abf.astype(bf),
            "af32": af32,
            "feats": np.ascontiguousarray(fb.reshape(BPC * T_LEN, L)),
            "oh": np.ascontiguousarray(oh[sl].reshape(BPC * (T_LEN + 1), L)),
        })
    return in_maps


def kernel(feats, tags, T):
    global _compiled
    from concourse.bass_utils import run_bass_kernel_spmd

    if _compiled is None:
        _compiled = _build_program()
    nc = _compiled

    in_maps = _prep_in_maps(feats, tags, T)
    res = run_bass_kernel_spmd(nc, in_maps, list(range(N_CORES)))
    outs = []
    for c in range(N_CORES):
        o = np.asarray(res.results[c]["out"], dtype=np.float64).reshape(2, BPC)
        outs.append(np.log(o[0]) + LN_OFF - o[1])
    return np.concatenate(outs).astype(np.float32)
